# revision 12
# baseline (speedup 1.0000x reference)
"""Trainium2 Bass kernel for ColorGNNEmbedding (3-layer GCN, N=50000, E=800000).

v2 design (post-trace): the v1 bottlenecks were DVE one-hot builds (5.4ms),
Q7 gather descriptor generation, and serial AllGathers. Changes:
  - S (one-hot x norm) matrices precomputed on host, streamed f16 from DRAM.
  - Embedding lookups folded on host into a per-node emb_sum tensor.
  - Edges partitioned by (dst tile, src piece) into NP=4 source pieces;
    the per-layer AllGather is split into 4 piece AllGathers that overlap
    with stage1 and with the aggregation passes of earlier pieces.
  - Gather padding indices are -1 (descgen skips trailing negatives).
  - Gathers are spread over 4 SWDGE queues.
  - L3 gather table stored 64 wide; fetches 256B spanning 2 rows
    (elem_step=64) to satisfy the 256B minimum row size.
Per GCN layer: stage1 matmul hW per piece -> AG piece -> agg pass per piece
(indirect gather + S-chunk matmuls accumulated in PSUM, merged into an SBUF
hpre accumulator) -> batchnorm stats via ones-matmul + AllReduce -> normalize
+ leaky relu.
"""

import math
import os
import numpy as np
from contextlib import ExitStack
from dataclasses import dataclass, field

P = 128
F_RES = 1000
F_IN = 1024
F1, F2, F3, FO = 512, 256, 64, 3
EPS = 1e-5
ALPHA = 0.01
NP_PIECES = 4


@dataclass
class Cfg:
    n: int = 50000
    n_cores: int = 8
    kp: list = field(default_factory=list)  # chunks per (tile, piece) pass

    @property
    def nloc(self):
        return self.n // self.n_cores

    @property
    def ntiles(self):
        return (self.nloc + P - 1) // P

    @property
    def npad(self):
        return self.ntiles * P

    @property
    def piece_tiles(self):
        nt, npz = self.ntiles, NP_PIECES
        base = nt // npz
        return [base + (1 if i < nt % npz else 0) for i in range(npz)]

    @property
    def piece_rows(self):
        return [t * P for t in self.piece_tiles]

    @property
    def piece_base(self):
        b = [0]
        for r in self.piece_rows:
            b.append(b[-1] + r)
        return b


# ---------------------------------------------------------------------------
# host-side preprocessing
# ---------------------------------------------------------------------------

def prep_edges(cfg: Cfg, edge_index, edge_attr):
    """Partition edges by (dst core, dst tile, src piece); compute kmax per
    piece; build per-core gather indices (wrapped int16, -1 padding) and
    precomputed S matrices [NT, P, Ktot*P] f16."""
    n, nloc, nt = cfg.n, cfg.nloc, cfg.ntiles
    npz = NP_PIECES
    prow, pbase = cfg.piece_rows, cfg.piece_base
    src = np.asarray(edge_index[0], np.int64)
    dst = np.asarray(edge_index[1], np.int64)
    ew = np.asarray(edge_attr, np.float64)
    deg = np.bincount(dst, weights=ew, minlength=n) + 1.0
    dis = 1.0 / np.sqrt(deg)
    w = dis[src] * ew * dis[dst]
    src_all = np.concatenate([src, np.arange(n)])
    dst_all = np.concatenate([dst, np.arange(n)])
    w_all = np.concatenate([w, 1.0 / deg]).astype(np.float32)

    iloc_src = src_all % nloc
    score = src_all // nloc
    piece = np.searchsorted(pbase, iloc_src, side="right") - 1
    # row of src in piece table p: score*prow[p] + (iloc - pbase[p])
    srow = score * np.asarray(prow)[piece] + (iloc_src - np.asarray(pbase)[piece])
    core = dst_all // nloc

    percore = []
    cnts = np.zeros((cfg.n_cores, nt, npz), np.int64)
    for c in range(cfg.n_cores):
        m = core == c
        d_loc = dst_all[m] - c * nloc
        tile, slot = d_loc // P, d_loc % P
        pc, sr, wv = piece[m], srow[m], w_all[m]
        grp = tile * npz + pc
        order = np.lexsort((sr, grp))
        tile, slot, pc, sr, wv, grp = (a[order] for a in (tile, slot, pc, sr, wv, grp))
        cnt = np.bincount(grp, minlength=nt * npz).reshape(nt, npz)
        cnts[c] = cnt
        percore.append((tile, slot, pc, sr, wv, grp, cnt))

    # per-(tile, piece) chunk counts: max over cores
    kpt = np.ceil(cnts.max(axis=0) / P).astype(np.int64)  # [nt, npz]
    cfg.kpt = kpt
    cfg.kp = [int(kpt[:, p].max()) for p in range(npz)]  # idx array widths
    koff_t = np.zeros((nt, npz + 1), np.int64)
    koff_t[:, 1:] = np.cumsum(kpt, axis=1)
    cfg.ktm = int(koff_t[:, -1].max())  # S tile width (chunks)

    packs = []
    for tile, slot, pc, sr, wv, grp, cnt in percore:
        starts = np.concatenate([[0], np.cumsum(cnt.reshape(-1))])[:-1]
        j = np.arange(len(sr)) - starts[grp]
        k, pos = j // P, j % P
        S = np.zeros((nt, P, cfg.ktm * P), np.float16)
        col = (koff_t[tile, pc] + k) * P + slot
        S[tile, pos, col] = wv
        # NOTE: -1 "skip" padding indices hang the device; pad with row 0.
        idxs = []
        for p in range(npz):
            unw = np.zeros((nt, cfg.kp[p] * P), np.int16)
            mp = pc == p
            unw[tile[mp], k[mp] * P + pos[mp]] = sr[mp].astype(np.int16)
            wrapped = unw.reshape(nt, cfg.kp[p] * 8, 16).transpose(0, 2, 1)
            idxs.append(np.ascontiguousarray(np.tile(wrapped, (1, 8, 1))))
        packs.append((S, idxs))
    return packs


def prep_nodes(cfg: Cfg, x, layer_emb, size_emb, color_emb, W1, c):
    """Per-core: transposed resnet tiles (stage-1 lhsT) and host-folded
    embedding contribution emb_sum = sum of table rows @ W1 blocks."""
    nloc, npad, nt = cfg.nloc, cfg.npad, cfg.ntiles
    xc = np.asarray(x[c * nloc:(c + 1) * nloc], np.float32)
    r = np.zeros((npad, F_IN), np.float16)
    r[:nloc, :F_RES] = xc[:, 1:1 + F_RES]
    r4 = r.reshape(nt, P, F_IN // P, P)
    resnet_t = np.ascontiguousarray(r4.transpose(0, 3, 2, 1)).reshape(nt, P, F_IN)

    W1 = np.asarray(W1, np.float64)
    le = np.asarray(layer_emb, np.float64) @ W1[0:250]
    se = np.asarray(size_emb, np.float64) @ W1[1250:1500]
    ce = np.asarray(color_emb, np.float64)
    cr = ce @ W1[1500:1585]
    cg = ce @ W1[1585:1670]
    cb = ce @ W1[1670:1755]
    i0 = xc[:, 0].astype(np.int32)
    i1 = np.rint(xc[:, 1001] * 10).astype(np.int32)
    ir = xc[:, 1002].astype(np.int32)
    ig = xc[:, 1003].astype(np.int32)
    ib = xc[:, 1004].astype(np.int32)
    es = le[i0] + se[i1] + cr[ir] + cg[ig] + cb[ib]
    emb = np.zeros((npad, F1), np.float16)
    emb[:nloc] = es.astype(np.float16)
    return resnet_t, np.ascontiguousarray(emb.reshape(nt, P, F1))


def prep_weights(W1, W2, W3, Wo, bo):
    w1p = np.zeros((F_IN, F1), np.float16)
    w1p[:F_RES] = np.asarray(W1, np.float32)[250:1250]
    w1r = np.ascontiguousarray(w1p.reshape(F_IN // P, P, F1))
    w2r = np.ascontiguousarray(np.asarray(W2, np.float16).reshape(F1 // P, P, F2))
    w3r = np.ascontiguousarray(np.asarray(W3, np.float16).reshape(F2 // P, P, F3))
    wor = np.ascontiguousarray(np.asarray(Wo, np.float16))
    bor = np.asarray(bo, np.float16).reshape(1, FO)
    return w1r, w2r, w3r, wor, bor


def prep_inputs(cfg: Cfg, inputs):
    x = np.asarray(inputs["x"], np.float32)
    packs = prep_edges(cfg, inputs["edge_index"], inputs["edge_attr"])
    w1r, w2r, w3r, wor, bor = prep_weights(
        inputs["W1"], inputs["W2"], inputs["W3"], inputs["Wo"], inputs["bo"])
    gb1 = np.ascontiguousarray(np.stack([inputs["g1"], inputs["be1"]]).astype(np.float32))
    gb2 = np.ascontiguousarray(np.stack([inputs["g2"], inputs["be2"]]).astype(np.float32))
    gb3 = np.ascontiguousarray(np.stack([inputs["g3"], inputs["be3"]]).astype(np.float32))

    in_maps = []
    for c in range(cfg.n_cores):
        resnet_t, emb_sum = prep_nodes(
            cfg, x, inputs["layer_emb"], inputs["size_emb"], inputs["color_emb"],
            inputs["W1"], c)
        S, idxs = packs[c]
        im = {
            "resnet": resnet_t, "embsum": emb_sum, "smat": S,
            "w1": w1r, "w2": w2r, "w3": w3r, "wo": wor, "wob": bor,
            "gb1": gb1, "gb2": gb2, "gb3": gb3,
            "ident": np.ascontiguousarray(np.eye(P, dtype=np.float16)),
        }
        for p in range(NP_PIECES):
            im[f"idx{p}"] = idxs[p]
        in_maps.append(im)
    return in_maps


# ---------------------------------------------------------------------------
# device program
# ---------------------------------------------------------------------------

def build_program(cfg: Cfg):
    import concourse.bacc as bacc
    import concourse.bass as bass
    import concourse.tile as tile
    from concourse import mybir

    f16, f32, i32, i16 = (mybir.dt.float16, mybir.dt.float32,
                          mybir.dt.int32, mybir.dt.int16)
    AF = mybir.ActivationFunctionType
    OP = mybir.AluOpType
    NT, NC, NPAD = cfg.ntiles, cfg.n_cores, cfg.npad
    NPZ = NP_PIECES
    KP = cfg.kp
    KOFF = [0]
    for k in KP:
        KOFF.append(KOFF[-1] + k)
    KTOT = KOFF[-1]
    PT = cfg.piece_tiles     # tiles per piece
    PR = cfg.piece_rows      # local rows per piece
    PB = cfg.piece_base
    TB = [b // P for b in PB]  # tile index base per piece
    GROUPS = [list(range(NC))]

    abl = set(os.environ.get("KABL", "").split(","))
    nc = bacc.Bacc("TRN2", target_bir_lowering=False, debug=False,
                   num_devices=NC, num_swdge_queues=4)

    # --- parameters -------------------------------------------------------
    resnet_d = nc.dram_tensor("resnet", [NT, P, F_IN], f16, kind="ExternalInput")
    embsum_d = nc.dram_tensor("embsum", [NT, P, F1], f16, kind="ExternalInput")
    smat_d = nc.dram_tensor("smat", [NT, P, KTOT * P], f16, kind="ExternalInput")
    idx_d = [nc.dram_tensor(f"idx{p}", [NT, P, KP[p] * 8], i16, kind="ExternalInput")
             for p in range(NPZ)]
    w1_d = nc.dram_tensor("w1", [F_IN // P, P, F1], f16, kind="ExternalInput")
    w2_d = nc.dram_tensor("w2", [F1 // P, P, F2], f16, kind="ExternalInput")
    w3_d = nc.dram_tensor("w3", [F2 // P, P, F3], f16, kind="ExternalInput")
    wo_d = nc.dram_tensor("wo", [64, FO], f16, kind="ExternalInput")
    wob_d = nc.dram_tensor("wob", [1, FO], f16, kind="ExternalInput")
    gb_d = [nc.dram_tensor(nm, [2, f], f32, kind="ExternalInput")
            for nm, f in (("gb1", F1), ("gb2", F2), ("gb3", F3))]
    ident_d = nc.dram_tensor("ident", [P, P], f16, kind="ExternalInput")
    out_d = nc.dram_tensor("out", [NPAD, FO], f32, kind="ExternalOutput")

    # --- internal DRAM ----------------------------------------------------
    Fs = [F1, F2, F3]          # layer output widths
    Wg = [F1, F2, P]           # gather table row widths (L3 zero-padded)
    FETCH = [F1, F2, P]        # gather elem sizes
    slab_d = [[nc.dram_tensor(f"slab{l}_{p}", [PR[p], Wg[l]], f16)
               for p in range(NPZ)] for l in range(3)]
    ft_d = [[nc.dram_tensor(f"ft{l}_{p}", [NC * PR[p], Wg[l]],
                            f16, addr_space="Shared")
             for p in range(NPZ)] for l in range(3)]
    sin_d = [nc.dram_tensor(f"sin{l}", [1, 2 * Fs[l]], f32) for l in range(3)]
    sout_d = [nc.dram_tensor(f"sout{l}", [1, 2 * Fs[l]], f32, addr_space="Shared")
              for l in range(3)]
    hnext_d = [nc.dram_tensor("h2d", [NPAD, F1], f16),
               nc.dram_tensor("h3d", [NPAD, F2], f16),
               nc.dram_tensor("h4d", [NPAD, P], f16)]

    with tile.TileContext(nc) as tc, ExitStack() as top:
        const = top.enter_context(tc.tile_pool(name="const", bufs=1))
        ones_col = const.tile([P, 1], f16)
        nc.vector.memset(ones_col[:], 1.0)
        ones_row = const.tile([1, P], f16)
        nc.vector.memset(ones_row[:], 1.0)

        w_sb = []
        for l, (wd, fin_t, fout) in enumerate(
                [(w1_d, F_IN // P, F1), (w2_d, F1 // P, F2), (w3_d, F2 // P, F3)]):
            wt = const.tile([P, fin_t * fout], f16, tag=f"w{l}")
            for a in range(fin_t):
                nc.sync.dma_start(out=wt[:, a * fout:(a + 1) * fout], in_=wd[a])
            w_sb.append(wt)
        wo_sb = const.tile([64, FO], f16)
        nc.sync.dma_start(out=wo_sb[:], in_=wo_d[:])
        wob_sb = const.tile([1, FO], f16)
        nc.sync.dma_start(out=wob_sb[:], in_=wob_d[:])
        ident_sb = const.tile([P, P], f16)
        nc.sync.dma_start(out=ident_sb[:], in_=ident_d[:])

        # gather indices, cached in SBUF for all three layers
        idx_sb = []
        for p in range(NPZ):
            it = const.tile([P, NT * KP[p] * 8], i16, tag=f"idx{p}")
            for t in range(NT):
                nc.sync.dma_start(
                    out=it[:, t * KP[p] * 8:(t + 1) * KP[p] * 8], in_=idx_d[p][t])
            idx_sb.append(it)

        def stats_and_norm(ctx, l, F, hpre, s_ps, q_ps, writer):
            """AllReduce sums/sumsq, compute scale/shift, normalize tiles."""
            sp = ctx.enter_context(tc.tile_pool(name=f"stat{l}", bufs=1))
            bp = ctx.enter_context(tc.tile_pool(name=f"statp{l}", bufs=2, space="PSUM"))
            ssb = sp.tile([1, 2 * F], f32)
            nc.vector.tensor_copy(ssb[:, 0:F], s_ps[:])
            nc.vector.tensor_copy(ssb[:, F:2 * F], q_ps[:])
            nc.sync.dma_start(out=sin_d[l][:], in_=ssb[:])
            nc.gpsimd.collective_compute(
                "AllReduce", OP.add, replica_groups=GROUPS,
                ins=[sin_d[l][:]], outs=[sout_d[l][:]])
            srep = sp.tile([1, 2 * F], f32)
            nc.sync.dma_start(out=srep[:], in_=sout_d[l][:])
            gsb = sp.tile([1, F], f32)
            nc.sync.dma_start(out=gsb[:], in_=gb_d[l][0:1, :])
            bsb = sp.tile([1, F], f32)
            nc.sync.dma_start(out=bsb[:], in_=gb_d[l][1:2, :])
            mean = sp.tile([1, F], f32)
            nc.vector.tensor_scalar_mul(mean[:], srep[:, 0:F], 1.0 / cfg.n)
            var = sp.tile([1, F], f32)
            nc.vector.tensor_scalar_mul(var[:], srep[:, F:2 * F], 1.0 / cfg.n)
            m2 = sp.tile([1, F], f32)
            nc.vector.tensor_tensor(out=m2[:], in0=mean[:], in1=mean[:], op=OP.mult)
            nc.vector.tensor_tensor(out=var[:], in0=var[:], in1=m2[:], op=OP.subtract)
            nc.vector.tensor_scalar_add(var[:], var[:], EPS)
            rec = sp.tile([1, F], f32)
            nc.vector.reciprocal(rec[:], var[:])
            rs = sp.tile([1, F], f32)
            nc.scalar.sqrt(rs[:], rec[:])
            sc = sp.tile([1, F], f32)
            nc.vector.tensor_tensor(out=sc[:], in0=gsb[:], in1=rs[:], op=OP.mult)
            sh = sp.tile([1, F], f32)
            nc.vector.tensor_tensor(out=sh[:], in0=mean[:], in1=sc[:], op=OP.mult)
            nc.vector.tensor_tensor(out=sh[:], in0=bsb[:], in1=sh[:], op=OP.subtract)
            sc16 = sp.tile([1, F], f16)
            nc.vector.tensor_copy(sc16[:], sc[:])
            sh16 = sp.tile([1, F], f16)
            nc.vector.tensor_copy(sh16[:], sh[:])
            scp = bp.tile([P, F], f32, space="PSUM")
            nc.tensor.matmul(out=scp[:], lhsT=ones_row[:], rhs=sc16[:],
                             start=True, stop=True)
            shp = bp.tile([P, F], f32, space="PSUM")
            nc.tensor.matmul(out=shp[:], lhsT=ones_row[:], rhs=sh16[:],
                             start=True, stop=True)
            screp = sp.tile([P, F], f16)
            nc.scalar.activation(screp[:], scp[:], AF.Copy)
            shrep = sp.tile([P, F], f16)
            nc.scalar.activation(shrep[:], shp[:], AF.Copy)

            np_pool = ctx.enter_context(tc.tile_pool(name=f"norm{l}", bufs=3))
            for t in range(NT):
                hn = np_pool.tile([P, F], f16, tag="hn")
                nc.vector.tensor_tensor(out=hn[:], in0=hpre[:, t * F:(t + 1) * F],
                                        in1=screp[:], op=OP.mult)
                nc.vector.tensor_tensor(out=hn[:], in0=hn[:], in1=shrep[:], op=OP.add)
                ha = np_pool.tile([P, F], f16, tag="ha")
                nc.scalar.activation(ha[:], hn[:], AF.Copy, scale=ALPHA)
                nc.vector.tensor_tensor(out=hn[:], in0=hn[:], in1=ha[:], op=OP.max)
                writer(t, hn)

        def agg_layer(ctx, actx, l, F):
            """Piece-pipelined edge aggregation -> hpre [P, NT*F] f16 + stats."""
            CW = FETCH[l]           # column stride per chunk in G
            KM = max(KP)
            hp_pool = ctx.enter_context(tc.tile_pool(name=f"hpre{l}", bufs=1))
            hpre = hp_pool.tile([P, NT * F], f16)
            sp_pool = ctx.enter_context(tc.tile_pool(name=f"aggsp{l}", bufs=2, space="PSUM"))
            s_ps = sp_pool.tile([1, F], f32)
            q_ps = sp_pool.tile([1, F], f32)
            ap_pool = actx.enter_context(tc.tile_pool(name=f"aggp{l}", bufs=4, space="PSUM"))
            g_pool = actx.enter_context(tc.tile_pool(name=f"g{l}", bufs=3))
            s_pool = actx.enter_context(tc.tile_pool(name=f"s{l}", bufs=3))
            sq_pool = actx.enter_context(tc.tile_pool(name=f"sq{l}", bufs=3))
            nbuf = 0
            for p in range(NPZ):
                K = KP[p]
                for t in range(NT):
                    G = g_pool.tile([P, KM * CW], f16, tag="g")
                    if nbuf < 3:
                        nbuf += 1
                        nc.vector.memset(G[:], 0.0)
                    if "nogather" not in abl:
                        nc.gpsimd.dma_gather(
                            out_ap=G[:, 0:K * CW].rearrange("q (k f) -> q k f", k=K),
                            in_ap=ft_d[l][p][:],
                            idxs_ap=idx_sb[p][:, t * K * 8:(t + 1) * K * 8],
                            num_idxs=K * P, num_idxs_reg=K * P, elem_size=CW,
                            single_packet=False,
                            queue_num=(p if "1q" not in abl else 0))
                    St = s_pool.tile([P, KM * P], f16, tag="s")
                    nc.sync.dma_start(
                        out=St[:, 0:K * P],
                        in_=smat_d[t][:, KOFF[p] * P:KOFF[p + 1] * P])
                    apsum = ap_pool.tile([P, F], f32, space="PSUM")
                    for k in range(K):
                        nc.tensor.matmul(out=apsum[:], lhsT=St[:, k * P:(k + 1) * P],
                                         rhs=G[:, k * CW:k * CW + F],
                                         start=(k == 0), stop=(k == K - 1))
                    hp = hpre[:, t * F:(t + 1) * F]
                    if p == 0:
                        nc.scalar.activation(hp, apsum[:], AF.Copy)
                    else:
                        a16 = sq_pool.tile([P, F], f16, tag="a16")
                        nc.scalar.activation(a16[:], apsum[:], AF.Copy)
                        nc.vector.tensor_tensor(out=hp, in0=a16[:], in1=hp, op=OP.add)
                    if p == NPZ - 1:
                        sq = sq_pool.tile([P, F], f16, tag="sq")
                        nc.scalar.square(sq[:], hp)
                        nc.tensor.matmul(out=s_ps[:], lhsT=ones_col[:], rhs=hp,
                                         start=(t == 0), stop=(t == NT - 1))
                        nc.tensor.matmul(out=q_ps[:], lhsT=ones_col[:], rhs=sq[:],
                                         start=(t == 0), stop=(t == NT - 1))
            return hpre, s_ps, q_ps

        def allgather_piece(l, p):
            if "noag" in abl:
                return
            nc.gpsimd.collective_compute(
                "AllGather", mybir.AluOpType.bypass, replica_groups=GROUPS,
                ins=[slab_d[l][p][:]],
                outs=[ft_d[l][p][0:NC * PR[p], :]])

        # ================= conv1 =================
        with ExitStack() as ctx:
            with nc.named_scope("stage1_c1"), ExitStack() as sctx:
                r_pool = sctx.enter_context(tc.tile_pool(name="res", bufs=3))
                e_pool = sctx.enter_context(tc.tile_pool(name="emb", bufs=3))
                p1_pool = sctx.enter_context(tc.tile_pool(name="p1", bufs=3, space="PSUM"))
                hw_pool = sctx.enter_context(tc.tile_pool(name="hw1", bufs=3))
                for p in range(NPZ):
                    for tt in range(PT[p]):
                        t = TB[p] + tt
                        rsb = r_pool.tile([P, F_IN], f16, tag="r")
                        nc.sync.dma_start(out=rsb[:], in_=resnet_d[t])
                        emb = e_pool.tile([P, F1], f16, tag="e")
                        nc.sync.dma_start(out=emb[:], in_=embsum_d[t])
                        ps = p1_pool.tile([P, F1], f32, space="PSUM")
                        for a in range(F_IN // P):
                            nc.tensor.matmul(
                                out=ps[:], lhsT=rsb[:, a * P:(a + 1) * P],
                                rhs=w_sb[0][:, a * F1:(a + 1) * F1],
                                start=(a == 0), stop=False)
                        nc.tensor.matmul(out=ps[:], lhsT=ident_sb[:], rhs=emb[:],
                                         start=False, stop=True)
                        hw = hw_pool.tile([P, F1], f16, tag="hw")
                        nc.scalar.activation(hw[:], ps[:], AF.Copy)
                        nc.sync.dma_start(
                            out=slab_d[0][p][tt * P:(tt + 1) * P, :], in_=hw[:])
                    with nc.named_scope(f"ag_c1_{p}"):
                        allgather_piece(0, p)
            with nc.named_scope("agg_c1"), ExitStack() as actx:
                hpre, s_ps, q_ps = agg_layer(ctx, actx, 0, F1)
            with nc.named_scope("bn_c1"):
                def w1out(t, hn):
                    nc.sync.dma_start(out=hnext_d[0][t * P:(t + 1) * P, :], in_=hn[:])
                stats_and_norm(ctx, 0, F1, hpre, s_ps, q_ps, w1out)

        # ================= conv2 =================
        with ExitStack() as ctx:
            ht_pool = ctx.enter_context(tc.tile_pool(name="h2T", bufs=1))
            h2T = ht_pool.tile([P, (F1 // P) * NPAD], f16)
            with nc.named_scope("tr_c2"):
                for j in range(F1 // P):
                    nc.sync.dma_start(
                        out=h2T[:, j * NPAD:(j + 1) * NPAD],
                        in_=hnext_d[0][:, j * P:(j + 1) * P], transpose=True)
            with nc.named_scope("stage1_c2"), ExitStack() as sctx:
                p2_pool = sctx.enter_context(tc.tile_pool(name="p2", bufs=3, space="PSUM"))
                hw_pool = sctx.enter_context(tc.tile_pool(name="hw2", bufs=3))
                for p in range(NPZ):
                    for tt in range(PT[p]):
                        t = TB[p] + tt
                        ps = p2_pool.tile([P, F2], f32, space="PSUM")
                        for a in range(F1 // P):
                            nc.tensor.matmul(
                                out=ps[:], lhsT=h2T[:, a * NPAD + t * P:a * NPAD + (t + 1) * P],
                                rhs=w_sb[1][:, a * F2:(a + 1) * F2],
                                start=(a == 0), stop=(a == F1 // P - 1))
                        hw = hw_pool.tile([P, F2], f16, tag="hw")
                        nc.scalar.activation(hw[:], ps[:], AF.Copy)
                        nc.sync.dma_start(
                            out=slab_d[1][p][tt * P:(tt + 1) * P, :], in_=hw[:])
                    with nc.named_scope(f"ag_c2_{p}"):
                        allgather_piece(1, p)
            with nc.named_scope("agg_c2"), ExitStack() as actx:
                hpre, s_ps, q_ps = agg_layer(ctx, actx, 1, F2)
            with nc.named_scope("bn_c2"):
                def w2out(t, hn):
                    nc.sync.dma_start(out=hnext_d[1][t * P:(t + 1) * P, :], in_=hn[:])
                stats_and_norm(ctx, 1, F2, hpre, s_ps, q_ps, w2out)

        # ================= conv3 =================
        with ExitStack() as ctx:
            ht_pool = ctx.enter_context(tc.tile_pool(name="h3T", bufs=1))
            h3T = ht_pool.tile([P, (F2 // P) * NPAD], f16)
            with nc.named_scope("tr_c3"):
                for j in range(F2 // P):
                    nc.sync.dma_start(
                        out=h3T[:, j * NPAD:(j + 1) * NPAD],
                        in_=hnext_d[1][:, j * P:(j + 1) * P], transpose=True)
            with nc.named_scope("stage1_c3"), ExitStack() as sctx:
                p3_pool = sctx.enter_context(tc.tile_pool(name="p3", bufs=3, space="PSUM"))
                hw_pool = sctx.enter_context(tc.tile_pool(name="hw3", bufs=3))
                for p in range(NPZ):
                    for tt in range(PT[p]):
                        t = TB[p] + tt
                        ps = p3_pool.tile([P, F3], f32, space="PSUM")
                        for a in range(F2 // P):
                            nc.tensor.matmul(
                                out=ps[:], lhsT=h3T[:, a * NPAD + t * P:a * NPAD + (t + 1) * P],
                                rhs=w_sb[2][:, a * F3:(a + 1) * F3],
                                start=(a == 0), stop=(a == F2 // P - 1))
                        hw = hw_pool.tile([P, P], f16, tag="hw")
                        nc.vector.memset(hw[:, F3:P], 0.0)
                        nc.scalar.activation(hw[:, 0:F3], ps[:], AF.Copy)
                        nc.sync.dma_start(
                            out=slab_d[2][p][tt * P:(tt + 1) * P, :], in_=hw[:])
                    with nc.named_scope(f"ag_c3_{p}"):
                        allgather_piece(2, p)
            with nc.named_scope("agg_c3"), ExitStack() as actx:
                hpre, s_ps, q_ps = agg_layer(ctx, actx, 2, F3)
            with nc.named_scope("bn_c3"):
                z_pool = ctx.enter_context(tc.tile_pool(name="h4z", bufs=1))
                h4z = z_pool.tile([P, P], f16)
                nc.vector.memset(h4z[:], 0.0)

                def w3out(t, hn):
                    nc.vector.tensor_copy(h4z[:, 0:F3], hn[:])
                    nc.sync.dma_start(out=hnext_d[2][t * P:(t + 1) * P, :], in_=h4z[:])
                stats_and_norm(ctx, 2, F3, hpre, s_ps, q_ps, w3out)

        # ================= output =================
        with ExitStack() as ctx, nc.named_scope("out"):
            ht_pool = ctx.enter_context(tc.tile_pool(name="h4T", bufs=1))
            h4T = ht_pool.tile([P, NPAD], f16)
            nc.sync.dma_start(out=h4T[:], in_=hnext_d[2][:], transpose=True)
            po_pool = ctx.enter_context(tc.tile_pool(name="po", bufs=4, space="PSUM"))
            o_pool = ctx.enter_context(tc.tile_pool(name="osb", bufs=4))
            for t in range(NT):
                ps = po_pool.tile([P, FO], f32, space="PSUM")
                nc.tensor.matmul(out=ps[:], lhsT=h4T[0:64, t * P:(t + 1) * P],
                                 rhs=wo_sb[:], start=True, stop=False)
                nc.tensor.matmul(out=ps[:], lhsT=ones_row[:], rhs=wob_sb[:],
                                 start=False, stop=True)
                ot = o_pool.tile([P, FO], f32)
                nc.scalar.activation(ot[:], ps[:], AF.Copy)
                nc.sync.dma_start(out=out_d[t * P:(t + 1) * P, :], in_=ot[:])

    nc.compile()
    return nc


# ---------------------------------------------------------------------------
# entry point
# ---------------------------------------------------------------------------

def run(inputs, cfg=None, trace=False):
    from concourse.bass_utils import run_bass_kernel_spmd

    if cfg is None:
        cfg = Cfg(n=int(np.asarray(inputs["x"]).shape[0]))
    in_maps = prep_inputs(cfg, inputs)
    nc = build_program(cfg)
    res = run_bass_kernel_spmd(nc, in_maps, core_ids=list(range(cfg.n_cores)),
                               trace=trace)
    out = np.empty((cfg.n, FO), np.float32)
    for c in range(cfg.n_cores):
        out[c * cfg.nloc:(c + 1) * cfg.nloc] = res.results[c]["out"][:cfg.nloc]
    return out, res


def kernel(**inputs) -> np.ndarray:
    out, _ = run(inputs)
    return out
